# revision 12
# baseline (speedup 1.0000x reference)
import sys
sys.path.insert(0, "/opt/trn_rl_repo")
import numpy as np
import ml_dtypes

BF16 = np.float16

N_ATOMS = 10000
N_SPECIES = 8
N_STRUCT = 8
C = 16
N_BASIS = 8
L_MAX = 3
CUTOFF = 5.0
NCORES = 8
NC_AT = N_ATOMS // NCORES

_prog_cache = {}


def _pack(senders, receivers):
    send = np.asarray(senders).astype(np.int64)
    recv = np.asarray(receivers).astype(np.int64)
    order = np.argsort(recv, kind="stable")
    ss = send[order]
    deg = np.bincount(recv, minlength=N_ATOMS)
    CNT = None
    for c in (8, 7, 6, 5, 4, 3, 2):
        ok = True
        npd = ((NC_AT + c - 1) // c) * c
        for core in range(NCORES):
            d = np.zeros(npd, np.int64)
            d[:NC_AT] = deg[core * NC_AT:(core + 1) * NC_AT]
            if d.reshape(-1, c).sum(1).max() > 128:
                ok = False
                break
        if ok:
            CNT = c
            break
    assert CNT is not None
    NCH = (NC_AT + CNT - 1) // CNT
    NCHE = NCH + (NCH & 1)
    NPAD = CNT * NCH
    starts = np.zeros(N_ATOMS + 1, np.int64)
    starts[1:] = np.cumsum(deg)
    cores = []
    for core in range(NCORES):
        slot_send = np.zeros((128, NCHE), np.int64)
        mask = np.zeros((128, NCHE, CNT), np.float32)
        valid = np.zeros((128, NCHE), bool)
        for k in range(NCH):
            row = 0
            for j in range(CNT):
                r = core * NC_AT + k * CNT + j
                if r >= (core + 1) * NC_AT:
                    continue
                a, b = starts[r], starts[r + 1]
                n = b - a
                slot_send[row:row + n, k] = ss[a:b]
                mask[row:row + n, k, j] = 1.0
                valid[row:row + n, k] = True
                row += n
            assert row <= 128
        cores.append(dict(slot_send=slot_send, mask=mask, valid=valid))
    return CNT, NCH, NCHE, NPAD, cores


def _build(CNT, NCH, NCHE, NPAD):
    import concourse.bass as bass
    import concourse.bacc as bacc
    import concourse.tile as tile
    from concourse import mybir

    f32 = mybir.dt.float32
    bf16 = mybir.dt.float16
    ALU = mybir.AluOpType
    AF = mybir.ActivationFunctionType

    NPADE = CNT * NCHE + CNT    # atom slots incl. pad chunk + view slack
    F = 16 * CNT                # (c, j) cols per chunk / pair
    PAIRS = NCHE // 2
    BP = max(1, 512 // F)       # pairs per block (PSUM bank = 512 f32)

    nc = bacc.Bacc("TRN2", target_bir_lowering=False, debug=False,
                   num_devices=NCORES)
    PP_d = nc.dram_tensor("pp", [128, 6, NCHE], f32, kind="ExternalInput").ap()
    MS_d = nc.dram_tensor("msk", [128, NCHE, CNT], bf16,
                          kind="ExternalInput").ap()
    HS_d = nc.dram_tensor("hs", [128, NCHE, 16], bf16,
                          kind="ExternalInput").ap()
    S2_d = nc.dram_tensor("s2", [128, 32], bf16, kind="ExternalInput").ap()
    W3_d = nc.dram_tensor("w3", [16, 16 * 17], bf16, kind="ExternalInput").ap()
    WR_d = nc.dram_tensor("wrb", [128, 8, 16], bf16, kind="ExternalInput").ap()
    CE_d = nc.dram_tensor("cemb", [16, NPAD], f32, kind="ExternalInput").ap()
    OUTH_d = nc.dram_tensor("outh", [16, NPAD], f32, kind="ExternalOutput").ap()
    OUTE_d = nc.dram_tensor("oute", [1, NPAD], f32, kind="ExternalOutput").ap()

    with tile.TileContext(nc) as tc:
        with tc.tile_pool(name="main", bufs=1) as pool, \
             tc.tile_pool(name="gp", bufs=3) as gpool, \
             tc.tile_pool(name="asp", bufs=3) as apool, \
             tc.tile_pool(name="pa", bufs=2, space="PSUM") as ppa, \
             tc.tile_pool(name="pi", bufs=2, space="PSUM") as ppi, \
             tc.tile_pool(name="ph", bufs=1, space="PSUM") as pph:
            PP = pool.tile([128, 6, NCHE], f32, tag="pp")
            S2 = pool.tile([128, 32], bf16, tag="s2")
            W3 = pool.tile([16, 16 * 17], bf16, tag="w3")
            WR = pool.tile([128, 8, 16], bf16, tag="wr")
            CE = pool.tile([16, NPAD], f32, tag="ce")
            MS = pool.tile([128, NCHE, CNT], bf16, tag="ms")
            HS = pool.tile([128, NCHE, 16], bf16, tag="hs")
            for t, d in ((PP, PP_d), (S2, S2_d), (W3, W3_d), (WR, WR_d),
                         (CE, CE_d), (MS, MS_d), (HS, HS_d)):
                nc.sync.dma_start(t[:], d[:])

            RV = pool.tile([128, 3, NCHE], f32, tag="rv")
            U = pool.tile([128, 3, NCHE], f32, tag="u")
            SC = pool.tile([128, 12, NCHE], f32, tag="sc")
            SHB = pool.tile([128, 16, NCHE], bf16, tag="shb")
            BS = pool.tile([128, 8, NCHE], bf16, tag="bs")
            RR = pool.tile([128, 16, NCHE], bf16, tag="rr")
            TM = pool.tile([128, 16, NCHE], bf16, tag="tm")
            PT = pool.tile([128, 64, NCHE], bf16, tag="pt")
            INV = pool.tile([16, 16, NPADE], bf16, tag="inv")

            def sc(i):
                return SC[:, i, :]

            TT = nc.vector.tensor_tensor
            TS = nc.vector.tensor_scalar

            # geometry (slot-major contiguous layouts)
            TT(RV[:], PP[:, 3:6, :], PP[:, 0:3, :], ALU.subtract)
            x, y, z = RV[:, 0, :], RV[:, 1, :], RV[:, 2, :]
            nc.vector.tensor_mul(sc(0), x, x)
            nc.vector.tensor_mul(sc(1), y, y)
            TT(sc(0), sc(0), sc(1), ALU.add)
            nc.vector.tensor_mul(sc(1), z, z)
            TT(sc(0), sc(0), sc(1), ALU.add)                 # r2
            nc.scalar.activation(sc(1), sc(0), AF.Sqrt)      # r
            nc.vector.tensor_scalar_max(sc(2), sc(1), 1e-6)  # rc
            nc.vector.reciprocal(sc(3), sc(2))               # rinv
            TT(U[:], RV[:],
               SC[:, 3:4, :].to_broadcast([128, 3, NCHE]), ALU.mult)
            # fc = 0.5*cos(pi*min(r,5)/5)+0.5 ; cos(x) = -sin(x - pi/2)
            nc.vector.tensor_scalar_min(sc(6), sc(1), CUTOFF)
            TS(sc(6), sc(6), float(np.pi / CUTOFF), float(-np.pi / 2),
               ALU.mult, ALU.add)
            nc.scalar.activation(sc(7), sc(6), AF.Sin)
            TS(sc(4), sc(7), -0.5, 0.5, ALU.mult, ALU.add)   # fc
            TT(sc(5), sc(4), sc(3), ALU.mult)
            nc.vector.tensor_scalar_mul(sc(5), sc(5),
                                        float(np.sqrt(2.0 / CUTOFF)))  # g
            x, y, z = U[:, 0, :], U[:, 1, :], U[:, 2, :]
            x2, y2, z2, xy, yz, xz = (sc(i) for i in (6, 7, 8, 9, 10, 11))
            nc.vector.tensor_mul(x2, x, x)
            nc.vector.tensor_mul(y2, y, y)
            nc.vector.tensor_mul(z2, z, z)
            nc.vector.tensor_mul(xy, x, y)
            nc.vector.tensor_mul(yz, y, z)
            nc.vector.tensor_mul(xz, x, z)
            d_, t_ = sc(0), sc(1)  # r2, r now dead

            def shm(m):
                return SHB[:, m, :]

            nc.vector.memset(shm(0), 0.28209479)
            nc.vector.tensor_scalar_mul(shm(1), y, 0.48860251)
            nc.vector.tensor_scalar_mul(shm(2), z, 0.48860251)
            nc.vector.tensor_scalar_mul(shm(3), x, 0.48860251)
            nc.vector.tensor_scalar_mul(shm(4), xy, 1.09254843)
            nc.vector.tensor_scalar_mul(shm(5), yz, 1.09254843)
            TS(shm(6), z2, 3.0 * 0.31539157, -0.31539157, ALU.mult, ALU.add)
            nc.vector.tensor_scalar_mul(shm(7), xz, 1.09254843)
            TT(d_, x2, y2, ALU.subtract)
            nc.vector.tensor_scalar_mul(shm(8), d_, 0.54627422)
            nc.vector.scalar_tensor_tensor(t_, x2, 3.0, y2, ALU.mult,
                                           ALU.subtract)
            TT(t_, t_, y, ALU.mult)
            nc.vector.tensor_scalar_mul(shm(9), t_, 0.59004359)
            TT(t_, xy, z, ALU.mult)
            nc.vector.tensor_scalar_mul(shm(10), t_, 2.89061144)
            TS(t_, z2, 5.0 * 0.45704579, -0.45704579, ALU.mult, ALU.add)
            TT(shm(11), t_, y, ALU.mult)
            TT(shm(13), t_, x, ALU.mult)
            TS(t_, z2, 5.0 * 0.37317633, -3.0 * 0.37317633, ALU.mult, ALU.add)
            TT(shm(12), t_, z, ALU.mult)
            TT(t_, d_, z, ALU.mult)
            nc.vector.tensor_scalar_mul(shm(14), t_, 1.44530572)
            nc.vector.scalar_tensor_tensor(t_, y2, -3.0, x2, ALU.mult, ALU.add)
            TT(t_, t_, x, ALU.mult)
            nc.vector.tensor_scalar_mul(shm(15), t_, 0.59004359)

            # bessel basis b=1..8 (times g), Chebyshev recurrence:
            # s1 = -sin(th-pi); 2cos = -2*sin(th-pi/2); s_b = 2cos*s_{b-1}-s_{b-2}
            C2, SA, SB, TP = sc(0), sc(1), sc(3), sc(4)
            TS(sc(6), sc(2), float(np.pi / CUTOFF), float(-np.pi),
               ALU.mult, ALU.add)
            nc.scalar.activation(sc(7), sc(6), AF.Sin)
            nc.vector.tensor_scalar_mul(SA, sc(7), -1.0)          # s1
            TS(sc(6), sc(2), float(np.pi / CUTOFF), float(-np.pi / 2),
               ALU.mult, ALU.add)
            nc.scalar.activation(sc(7), sc(6), AF.Sin)
            nc.vector.tensor_scalar_mul(C2, sc(7), -2.0)          # 2cos
            for b in range(1, N_BASIS + 1):
                if b == 1:
                    cur = SA
                elif b == 2:
                    TT(SB, C2, SA, ALU.mult)
                    cur = SB
                else:
                    TT(TP, C2, SB if b % 2 else SA, ALU.mult)
                    dst = SA if b % 2 else SB
                    TT(dst, TP, SA if b % 2 else SB, ALU.subtract)
                    cur = dst
                TT(BS[:, b - 1, :], cur, sc(5), ALU.mult)   # bf16 out

            # RR[ln, k] = sum_b BS[b, k] * WR[b, ln]   (ln-major, fp16)
            # split across vector (even b) and gpsimd (odd b)
            RRG = pool.tile([128, 16, NCHE], bf16, tag="rrg")
            TMG = pool.tile([128, 16, NCHE], bf16, tag="tmg")
            GT = nc.gpsimd.tensor_tensor
            for b in range(N_BASIS):
                bsb = BS[:, b:b + 1, :].to_broadcast([128, 16, NCHE])
                wrb = WR[:, b, :].unsqueeze(2).to_broadcast([128, 16, NCHE])
                if b == 0:
                    TT(RR[:], bsb, wrb, ALU.mult)
                elif b == 1:
                    GT(RRG[:], bsb, wrb, ALU.mult)
                elif b % 2 == 0:
                    TT(TM[:], bsb, wrb, ALU.mult)
                    TT(RR[:], RR[:], TM[:], ALU.add)
                else:
                    GT(TMG[:], bsb, wrb, ALU.mult)
                    GT(RRG[:], RRG[:], TMG[:], ALU.add)
            TT(RR[:], RR[:], RRG[:], ALU.add)

            # PT[(m,n), k] = SHB[m, k] * RR[l(m)n, k]  (k-inner, contiguous)
            for l in range(L_MAX + 1):
                nm = 2 * l + 1
                sh_v = SHB[:, l * l:l * l + nm, :].unsqueeze(2).to_broadcast(
                    [128, nm, 4, NCHE])
                rr_v = RR[:, l * 4:l * 4 + 4, :].unsqueeze(1).to_broadcast(
                    [128, nm, 4, NCHE])
                pt_v = PT[:, l * l * 4:(l * l + nm) * 4, :].rearrange(
                    "p (m n) k -> p m n k", n=4)
                TT(pt_v, sh_v, rr_v, ALU.mult)

            # chunk blocks: G -> pa matmuls -> square -> S2 matmul -> INV
            p0 = 0
            while p0 < PAIRS:
                nbp = min(BP, PAIRS - p0)
                nck = 2 * nbp
                k0 = 2 * p0
                G = gpool.tile([128, 2 * BP, 16, CNT], bf16, tag="g")
                gop = TT if (p0 // BP) % 2 == 0 else nc.gpsimd.tensor_tensor
                gop(G[:, 0:nck, :, :],
                   HS[:, k0:k0 + nck, :].unsqueeze(3).to_broadcast(
                       [128, nck, 16, CNT]),
                   MS[:, k0:k0 + nck, :].unsqueeze(2).to_broadcast(
                       [128, nck, 16, CNT]),
                   ALU.mult)
                pa = ppa.tile([128, 512], f32, tag="pa")
                for p in range(nbp):
                    for h in (0, 1):
                        kk = 2 * p + h
                        nc.tensor.matmul(
                            pa[64 * h:64 * (h + 1), F * p:F * (p + 1)],
                            PT[:, :, k0 + kk],
                            G[:, kk, :, :],
                            start=True, stop=True)
                As = apool.tile([128, 512], bf16, tag="as")
                nc.scalar.activation(As[:, 0:F * nbp], pa[:, 0:F * nbp],
                                     AF.Square)
                # INV[ln, c, a] (c-major); chunk kk atoms at a=(k0+kk)*CNT
                for h in (0, 1):
                    pi = ppi.tile([16, 512], f32, tag="pi")
                    nc.tensor.matmul(pi[:, 0:F * nbp],
                                     S2[:, 16 * h:16 * (h + 1)],
                                     As[:, 0:F * nbp], start=True, stop=True)
                    src = pi[:, 0:F * nbp].rearrange(
                        "p (q c j) -> p q c j", c=16, j=CNT)
                    a0 = (k0 + h) * CNT
                    dst = INV[:, :, a0:a0 + nbp * 2 * CNT].rearrange(
                        "p c (q r) -> p q c r", r=2 * CNT)[:, :, :, 0:CNT]
                    nc.scalar.copy(dst, src)
                p0 += nbp

            # W apply: ph[o, a] = sum_c W3[:, (c,:)] @ INV[:, c, a]
            OUTS = pool.tile([16, NPAD], f32, tag="outs")
            GRP = 512
            ngrp = (NPAD + GRP - 1) // GRP
            phs = [pph.tile([17, 512], f32, tag=f"ph{gi}", name=f"ph{gi}")
                   for gi in range(ngrp)]
            for c in range(16):
                for gi in range(ngrp):
                    off = gi * GRP
                    gsz = min(GRP, NPAD - off)
                    nc.tensor.matmul(phs[gi][:, 0:gsz],
                                     W3[:, c * 17:(c + 1) * 17],
                                     INV[:, c, off:off + gsz],
                                     start=(c == 0), stop=(c == 15))
            ES = pool.tile([17, NPAD], f32, tag="es")
            for gi in range(ngrp):
                off = gi * GRP
                gsz = min(GRP, NPAD - off)
                nc.scalar.copy(ES[:, off:off + gsz], phs[gi][:, 0:gsz])
                nc.vector.tensor_mul(OUTS[:, off:off + gsz],
                                     ES[0:16, off:off + gsz],
                                     CE[:, off:off + gsz])
            nc.sync.dma_start(OUTH_d[:], OUTS[:])
            nc.sync.dma_start(OUTE_d[:], ES[16:17, :])
    nc.compile()
    return nc


def kernel(positions, embed, W_rad, W_inv1, W_inv2, w_out, comp_weights,
           senders, receivers, species, structure_ids):
    from concourse import bass_utils

    positions = np.asarray(positions, np.float32)
    embed = np.asarray(embed, np.float32)
    W_rad = np.asarray(W_rad, np.float32)
    W_inv1 = np.asarray(W_inv1, np.float32)
    W_inv2 = np.asarray(W_inv2, np.float32)
    w_out = np.asarray(w_out, np.float32)
    comp_weights = np.asarray(comp_weights, np.float32)
    senders = np.asarray(senders).astype(np.int64)
    receivers = np.asarray(receivers).astype(np.int64)
    species = np.asarray(species).astype(np.int64)
    structure_ids_np = np.asarray(structure_ids).astype(np.int64)

    CNT, NCH, NCHE, NPAD, cores = _pack(senders, receivers)
    key = (CNT, NCH)
    if key not in _prog_cache:
        _prog_cache[key] = _build(CNT, NCH, NCHE, NPAD)
    nc = _prog_cache[key]

    cemb = embed[species]  # [N,16]
    # binary S2 selector (1/sqrt(2l+1) folded into W3)
    S2 = np.zeros((128, 32), np.float32)
    mi = 0
    for l in range(L_MAX + 1):
        for m in range(2 * l + 1):
            for n in range(4):
                for h in (0, 1):
                    S2[h * 64 + mi * 4 + n, h * 16 + l * 4 + n] = 1.0
            mi += 1
    S2 = S2.astype(BF16)
    WRB = np.zeros((8, 16), np.float32)
    for l in range(L_MAX + 1):
        WRB[:, l * 4:(l + 1) * 4] = W_rad[l]
    WRB = np.broadcast_to(WRB[None], (128, 8, 16)).astype(BF16).copy()

    def w3_pack(W, wo, hscale):
        Waug = np.concatenate([W * hscale, wo[:, None]], 1)  # [256,17]
        W3 = np.zeros((16, 16 * 17), np.float32)
        for l in range(L_MAX + 1):
            s = 1.0 / np.sqrt(2.0 * l + 1.0)
            for n in range(4):
                for c in range(16):
                    W3[l * 4 + n, c * 17:(c + 1) * 17] = \
                        Waug[(l * 4 + n) * 16 + c] * s
        return W3.astype(BF16)

    base_maps = []
    for core in range(NCORES):
        cd = cores[core]
        ss, msk, val = cd["slot_send"], cd["mask"], cd["valid"]
        pp = np.zeros((128, 6, NCHE), np.float32)
        rloc = msk.argmax(2)
        rglob = core * NC_AT + (np.arange(NCHE)[None, :] * CNT + rloc)
        rglob = np.clip(rglob, 0, N_ATOMS - 1)
        pp[:, 0:3, :] = np.where(val[:, None, :],
                                 positions[ss].transpose(0, 2, 1), 0.0)
        pp[:, 3:6, :] = np.where(val[:, None, :],
                                 positions[rglob].transpose(0, 2, 1), 0.0)
        at = np.arange(core * NC_AT, core * NC_AT + NPAD)
        atc = np.clip(at, 0, N_ATOMS - 1)
        apad = (at < N_ATOMS)
        cemb_t = np.where(apad[None, :], cemb[atc].T, 0.0).astype(np.float32)
        base_maps.append(dict(pp=pp, msk=msk.astype(BF16), s2=S2, wrb=WRB,
                              cemb=np.ascontiguousarray(cemb_t)))

    def run(hsrc, w3):
        maps = []
        for core in range(NCORES):
            cd = cores[core]
            hsl = np.where(cd["valid"][:, :, None],
                           hsrc[cd["slot_send"]], 0.0).astype(BF16)
            maps.append(dict(base_maps[core], hs=hsl, w3=w3))
        return bass_utils.run_bass_kernel_spmd(nc, maps,
                                               core_ids=list(range(NCORES)))

    HSC = 1.0 / 16.0
    res1 = run(cemb, w3_pack(W_inv1, np.zeros(256, np.float32), HSC))
    h1 = np.concatenate(
        [res1.results[c]["outh"][:, 0:NC_AT].T for c in range(NCORES)], 0)

    res2 = run(h1, w3_pack(W_inv2, w_out / (HSC * HSC), 1.0))
    e_atom = np.concatenate(
        [res2.results[c]["oute"][0, 0:NC_AT] for c in range(NCORES)], 0)
    e_atom = e_atom + comp_weights[species]

    out = np.zeros(N_STRUCT, np.float32)
    np.add.at(out, structure_ids_np, e_atom)
    return out


# revision 13
# speedup vs baseline: 1.1346x; 1.1346x over previous
import sys
sys.path.insert(0, "/opt/trn_rl_repo")
import numpy as np
import ml_dtypes

BF16 = np.float16

N_ATOMS = 10000
N_SPECIES = 8
N_STRUCT = 8
C = 16
N_BASIS = 8
L_MAX = 3
CUTOFF = 5.0
NCORES = 8
NC_AT = N_ATOMS // NCORES

_prog_cache = {}


def _pack(senders, receivers):
    send = np.asarray(senders).astype(np.int64)
    recv = np.asarray(receivers).astype(np.int64)
    order = np.argsort(recv, kind="stable")
    ss = send[order]
    deg = np.bincount(recv, minlength=N_ATOMS)
    CNT = None
    for c in (8, 7, 6, 5, 4, 3, 2):
        ok = True
        npd = ((NC_AT + c - 1) // c) * c
        for core in range(NCORES):
            d = np.zeros(npd, np.int64)
            d[:NC_AT] = deg[core * NC_AT:(core + 1) * NC_AT]
            if d.reshape(-1, c).sum(1).max() > 128:
                ok = False
                break
        if ok:
            CNT = c
            break
    assert CNT is not None
    NCH = (NC_AT + CNT - 1) // CNT
    NCHE = NCH + (NCH & 1)
    NPAD = CNT * NCH
    starts = np.zeros(N_ATOMS + 1, np.int64)
    starts[1:] = np.cumsum(deg)
    cores = []
    for core in range(NCORES):
        slot_send = np.zeros((128, NCHE), np.int64)
        mask = np.zeros((128, NCHE, CNT), np.float32)
        valid = np.zeros((128, NCHE), bool)
        for k in range(NCH):
            row = 0
            for j in range(CNT):
                r = core * NC_AT + k * CNT + j
                if r >= (core + 1) * NC_AT:
                    continue
                a, b = starts[r], starts[r + 1]
                n = b - a
                slot_send[row:row + n, k] = ss[a:b]
                mask[row:row + n, k, j] = 1.0
                valid[row:row + n, k] = True
                row += n
            assert row <= 128
        cores.append(dict(slot_send=slot_send, mask=mask, valid=valid))
    return CNT, NCH, NCHE, NPAD, cores


def _build(CNT, NCH, NCHE, NPAD):
    import concourse.bass as bass
    import concourse.bacc as bacc
    import concourse.tile as tile
    from concourse import mybir

    f32 = mybir.dt.float32
    bf16 = mybir.dt.float16
    ALU = mybir.AluOpType
    AF = mybir.ActivationFunctionType

    NPADE = CNT * NCHE + CNT    # atom slots incl. pad chunk + view slack
    F = 16 * CNT                # (c, j) cols per chunk / pair
    PAIRS = NCHE // 2
    BP = max(1, 512 // F)       # pairs per block (PSUM bank = 512 f32)

    nc = bacc.Bacc("TRN2", target_bir_lowering=False, debug=False,
                   num_devices=NCORES)
    PP_d = nc.dram_tensor("pp", [128, 6, NCHE], f32, kind="ExternalInput").ap()
    MS_d = nc.dram_tensor("msk", [128, NCHE, CNT], bf16,
                          kind="ExternalInput").ap()
    HS_d = nc.dram_tensor("hs", [128, NCHE, 16], bf16,
                          kind="ExternalInput").ap()
    S2_d = nc.dram_tensor("s2", [128, 32], bf16, kind="ExternalInput").ap()
    W3_d = nc.dram_tensor("w3", [16, 16 * 17], bf16, kind="ExternalInput").ap()
    WR_d = nc.dram_tensor("wrb", [128, 8, 16], bf16, kind="ExternalInput").ap()
    CE_d = nc.dram_tensor("cemb", [16, NPAD], f32, kind="ExternalInput").ap()
    OUTH_d = nc.dram_tensor("outh", [16, NPAD], f32, kind="ExternalOutput").ap()
    OUTE_d = nc.dram_tensor("oute", [1, NPAD], f32, kind="ExternalOutput").ap()

    with tile.TileContext(nc) as tc:
        with tc.tile_pool(name="main", bufs=1) as pool, \
             tc.tile_pool(name="gp", bufs=3) as gpool, \
             tc.tile_pool(name="asp", bufs=3) as apool, \
             tc.tile_pool(name="pa", bufs=2, space="PSUM") as ppa, \
             tc.tile_pool(name="pi", bufs=2, space="PSUM") as ppi, \
             tc.tile_pool(name="ph", bufs=1, space="PSUM") as pph:
            PP = pool.tile([128, 6, NCHE], f32, tag="pp")
            S2 = pool.tile([128, 32], bf16, tag="s2")
            W3 = pool.tile([16, 16 * 17], bf16, tag="w3")
            WR = pool.tile([128, 8, 16], bf16, tag="wr")
            CE = pool.tile([16, NPAD], f32, tag="ce")
            MS = pool.tile([128, NCHE, CNT], bf16, tag="ms")
            HS = pool.tile([128, NCHE, 16], bf16, tag="hs")
            for t, d in ((PP, PP_d), (S2, S2_d), (W3, W3_d), (WR, WR_d),
                         (CE, CE_d), (MS, MS_d), (HS, HS_d)):
                nc.sync.dma_start(t[:], d[:])

            RV = pool.tile([128, 3, NCHE], f32, tag="rv")
            U = pool.tile([128, 3, NCHE], f32, tag="u")
            SC = pool.tile([128, 12, NCHE], f32, tag="sc")
            SHB = pool.tile([128, 16, NCHE], bf16, tag="shb")
            BS = pool.tile([128, 8, NCHE], bf16, tag="bs")
            RR = pool.tile([128, NCHE, 16], bf16, tag="rr")
            TM = pool.tile([128, NCHE, 16], bf16, tag="tm")
            PT = pool.tile([128, NCHE, 64], bf16, tag="pt")
            INV = pool.tile([16, 16, NPADE], bf16, tag="inv")

            def sc(i):
                return SC[:, i, :]

            TT = nc.vector.tensor_tensor
            TS = nc.vector.tensor_scalar

            # geometry (slot-major contiguous layouts)
            TT(RV[:], PP[:, 3:6, :], PP[:, 0:3, :], ALU.subtract)
            x, y, z = RV[:, 0, :], RV[:, 1, :], RV[:, 2, :]
            nc.vector.tensor_mul(sc(0), x, x)
            nc.vector.tensor_mul(sc(1), y, y)
            TT(sc(0), sc(0), sc(1), ALU.add)
            nc.vector.tensor_mul(sc(1), z, z)
            TT(sc(0), sc(0), sc(1), ALU.add)                 # r2
            nc.scalar.activation(sc(1), sc(0), AF.Sqrt)      # r
            nc.vector.tensor_scalar_max(sc(2), sc(1), 1e-6)  # rc
            nc.vector.reciprocal(sc(3), sc(2))               # rinv
            TT(U[:], RV[:],
               SC[:, 3:4, :].to_broadcast([128, 3, NCHE]), ALU.mult)
            # fc = 0.5*cos(pi*min(r,5)/5)+0.5 ; cos(x) = -sin(x - pi/2)
            nc.vector.tensor_scalar_min(sc(6), sc(1), CUTOFF)
            TS(sc(6), sc(6), float(np.pi / CUTOFF), float(-np.pi / 2),
               ALU.mult, ALU.add)
            nc.scalar.activation(sc(7), sc(6), AF.Sin)
            TS(sc(4), sc(7), -0.5, 0.5, ALU.mult, ALU.add)   # fc
            TT(sc(5), sc(4), sc(3), ALU.mult)
            nc.vector.tensor_scalar_mul(sc(5), sc(5),
                                        float(np.sqrt(2.0 / CUTOFF)))  # g
            x, y, z = U[:, 0, :], U[:, 1, :], U[:, 2, :]
            x2, y2, z2, xy, yz, xz = (sc(i) for i in (6, 7, 8, 9, 10, 11))
            nc.vector.tensor_mul(x2, x, x)
            nc.vector.tensor_mul(y2, y, y)
            nc.vector.tensor_mul(z2, z, z)
            nc.vector.tensor_mul(xy, x, y)
            nc.vector.tensor_mul(yz, y, z)
            nc.vector.tensor_mul(xz, x, z)
            d_, t_ = sc(0), sc(1)  # r2, r now dead

            def shm(m):
                return SHB[:, m, :]

            nc.vector.memset(shm(0), 0.28209479)
            nc.vector.tensor_scalar_mul(shm(1), y, 0.48860251)
            nc.vector.tensor_scalar_mul(shm(2), z, 0.48860251)
            nc.vector.tensor_scalar_mul(shm(3), x, 0.48860251)
            nc.vector.tensor_scalar_mul(shm(4), xy, 1.09254843)
            nc.vector.tensor_scalar_mul(shm(5), yz, 1.09254843)
            TS(shm(6), z2, 3.0 * 0.31539157, -0.31539157, ALU.mult, ALU.add)
            nc.vector.tensor_scalar_mul(shm(7), xz, 1.09254843)
            TT(d_, x2, y2, ALU.subtract)
            nc.vector.tensor_scalar_mul(shm(8), d_, 0.54627422)
            nc.vector.scalar_tensor_tensor(t_, x2, 3.0, y2, ALU.mult,
                                           ALU.subtract)
            TT(t_, t_, y, ALU.mult)
            nc.vector.tensor_scalar_mul(shm(9), t_, 0.59004359)
            TT(t_, xy, z, ALU.mult)
            nc.vector.tensor_scalar_mul(shm(10), t_, 2.89061144)
            TS(t_, z2, 5.0 * 0.45704579, -0.45704579, ALU.mult, ALU.add)
            TT(shm(11), t_, y, ALU.mult)
            TT(shm(13), t_, x, ALU.mult)
            TS(t_, z2, 5.0 * 0.37317633, -3.0 * 0.37317633, ALU.mult, ALU.add)
            TT(shm(12), t_, z, ALU.mult)
            TT(t_, d_, z, ALU.mult)
            nc.vector.tensor_scalar_mul(shm(14), t_, 1.44530572)
            nc.vector.scalar_tensor_tensor(t_, y2, -3.0, x2, ALU.mult, ALU.add)
            TT(t_, t_, x, ALU.mult)
            nc.vector.tensor_scalar_mul(shm(15), t_, 0.59004359)

            # bessel basis b=1..8 (times g), Chebyshev recurrence:
            # s1 = -sin(th-pi); 2cos = -2*sin(th-pi/2); s_b = 2cos*s_{b-1}-s_{b-2}
            C2, SA, SB, TP = sc(0), sc(1), sc(3), sc(4)
            TS(sc(6), sc(2), float(np.pi / CUTOFF), float(-np.pi),
               ALU.mult, ALU.add)
            nc.scalar.activation(sc(7), sc(6), AF.Sin)
            nc.vector.tensor_scalar_mul(SA, sc(7), -1.0)          # s1
            TS(sc(6), sc(2), float(np.pi / CUTOFF), float(-np.pi / 2),
               ALU.mult, ALU.add)
            nc.scalar.activation(sc(7), sc(6), AF.Sin)
            nc.vector.tensor_scalar_mul(C2, sc(7), -2.0)          # 2cos
            for b in range(1, N_BASIS + 1):
                if b == 1:
                    cur = SA
                elif b == 2:
                    TT(SB, C2, SA, ALU.mult)
                    cur = SB
                else:
                    TT(TP, C2, SB if b % 2 else SA, ALU.mult)
                    dst = SA if b % 2 else SB
                    TT(dst, TP, SA if b % 2 else SB, ALU.subtract)
                    cur = dst
                TT(BS[:, b - 1, :], cur, sc(5), ALU.mult)   # bf16 out

            # RR[k, ln] = sum_b BS[b, k] * WR[b, ln]   (k-major, fp16)
            for b in range(N_BASIS):
                bsb = BS[:, b, :].unsqueeze(2).to_broadcast([128, NCHE, 16])
                wrb = WR[:, b, :].unsqueeze(1).to_broadcast([128, NCHE, 16])
                if b == 0:
                    TT(RR[:], bsb, wrb, ALU.mult)
                else:
                    TT(TM[:], bsb, wrb, ALU.mult)
                    TT(RR[:], RR[:], TM[:], ALU.add)

            # PT[k, (m,n)] = SHB[m, k] * RR[k, l(m)n]
            for l in range(L_MAX + 1):
                nm = 2 * l + 1
                sh_v = SHB[:, l * l:l * l + nm, :].rearrange(
                    "p m k -> p k m").unsqueeze(3).to_broadcast(
                    [128, NCHE, nm, 4])
                rr_v = RR[:, :, l * 4:l * 4 + 4].unsqueeze(2).to_broadcast(
                    [128, NCHE, nm, 4])
                pt_v = PT[:, :, l * l * 4:(l * l + nm) * 4].rearrange(
                    "p k (m n) -> p k m n", n=4)
                TT(pt_v, sh_v, rr_v, ALU.mult)

            # chunk blocks: G -> pa matmuls -> square -> S2 matmul -> INV
            p0 = 0
            while p0 < PAIRS:
                nbp = min(BP, PAIRS - p0)
                nck = 2 * nbp
                k0 = 2 * p0
                G = gpool.tile([128, 2 * BP, 16, CNT], bf16, tag="g")
                nc.gpsimd.tensor_tensor(G[:, 0:nck, :, :],
                   HS[:, k0:k0 + nck, :].unsqueeze(3).to_broadcast(
                       [128, nck, 16, CNT]),
                   MS[:, k0:k0 + nck, :].unsqueeze(2).to_broadcast(
                       [128, nck, 16, CNT]),
                   ALU.mult)
                pa = ppa.tile([128, 512], f32, tag="pa")
                for p in range(nbp):
                    for h in (0, 1):
                        kk = 2 * p + h
                        nc.tensor.matmul(
                            pa[64 * h:64 * (h + 1), F * p:F * (p + 1)],
                            PT[:, k0 + kk, :],
                            G[:, kk, :, :],
                            start=True, stop=True)
                As = apool.tile([128, 512], bf16, tag="as")
                nc.scalar.activation(As[:, 0:F * nbp], pa[:, 0:F * nbp],
                                     AF.Square)
                # INV[ln, c, a] (c-major); chunk kk atoms at a=(k0+kk)*CNT
                for h in (0, 1):
                    pi = ppi.tile([16, 512], f32, tag="pi")
                    nc.tensor.matmul(pi[:, 0:F * nbp],
                                     S2[:, 16 * h:16 * (h + 1)],
                                     As[:, 0:F * nbp], start=True, stop=True)
                    src = pi[:, 0:F * nbp].rearrange(
                        "p (q c j) -> p q c j", c=16, j=CNT)
                    a0 = (k0 + h) * CNT
                    dst = INV[:, :, a0:a0 + nbp * 2 * CNT].rearrange(
                        "p c (q r) -> p q c r", r=2 * CNT)[:, :, :, 0:CNT]
                    nc.scalar.copy(dst, src)
                p0 += nbp

            # W apply: ph[o, a] = sum_c W3[:, (c,:)] @ INV[:, c, a]
            OUTS = pool.tile([16, NPAD], f32, tag="outs")
            GRP = 512
            ngrp = (NPAD + GRP - 1) // GRP
            phs = [pph.tile([17, 512], f32, tag=f"ph{gi}", name=f"ph{gi}")
                   for gi in range(ngrp)]
            for c in range(16):
                for gi in range(ngrp):
                    off = gi * GRP
                    gsz = min(GRP, NPAD - off)
                    nc.tensor.matmul(phs[gi][:, 0:gsz],
                                     W3[:, c * 17:(c + 1) * 17],
                                     INV[:, c, off:off + gsz],
                                     start=(c == 0), stop=(c == 15))
            ES = pool.tile([17, NPAD], f32, tag="es")
            for gi in range(ngrp):
                off = gi * GRP
                gsz = min(GRP, NPAD - off)
                nc.scalar.copy(ES[:, off:off + gsz], phs[gi][:, 0:gsz])
                nc.vector.tensor_mul(OUTS[:, off:off + gsz],
                                     ES[0:16, off:off + gsz],
                                     CE[:, off:off + gsz])
            nc.sync.dma_start(OUTH_d[:], OUTS[:])
            nc.sync.dma_start(OUTE_d[:], ES[16:17, :])
    nc.compile()
    return nc


def kernel(positions, embed, W_rad, W_inv1, W_inv2, w_out, comp_weights,
           senders, receivers, species, structure_ids):
    from concourse import bass_utils

    positions = np.asarray(positions, np.float32)
    embed = np.asarray(embed, np.float32)
    W_rad = np.asarray(W_rad, np.float32)
    W_inv1 = np.asarray(W_inv1, np.float32)
    W_inv2 = np.asarray(W_inv2, np.float32)
    w_out = np.asarray(w_out, np.float32)
    comp_weights = np.asarray(comp_weights, np.float32)
    senders = np.asarray(senders).astype(np.int64)
    receivers = np.asarray(receivers).astype(np.int64)
    species = np.asarray(species).astype(np.int64)
    structure_ids_np = np.asarray(structure_ids).astype(np.int64)

    CNT, NCH, NCHE, NPAD, cores = _pack(senders, receivers)
    key = (CNT, NCH)
    if key not in _prog_cache:
        _prog_cache[key] = _build(CNT, NCH, NCHE, NPAD)
    nc = _prog_cache[key]

    cemb = embed[species]  # [N,16]
    # binary S2 selector (1/sqrt(2l+1) folded into W3)
    S2 = np.zeros((128, 32), np.float32)
    mi = 0
    for l in range(L_MAX + 1):
        for m in range(2 * l + 1):
            for n in range(4):
                for h in (0, 1):
                    S2[h * 64 + mi * 4 + n, h * 16 + l * 4 + n] = 1.0
            mi += 1
    S2 = S2.astype(BF16)
    WRB = np.zeros((8, 16), np.float32)
    for l in range(L_MAX + 1):
        WRB[:, l * 4:(l + 1) * 4] = W_rad[l]
    WRB = np.broadcast_to(WRB[None], (128, 8, 16)).astype(BF16).copy()

    def w3_pack(W, wo, hscale):
        Waug = np.concatenate([W * hscale, wo[:, None]], 1)  # [256,17]
        W3 = np.zeros((16, 16 * 17), np.float32)
        for l in range(L_MAX + 1):
            s = 1.0 / np.sqrt(2.0 * l + 1.0)
            for n in range(4):
                for c in range(16):
                    W3[l * 4 + n, c * 17:(c + 1) * 17] = \
                        Waug[(l * 4 + n) * 16 + c] * s
        return W3.astype(BF16)

    base_maps = []
    for core in range(NCORES):
        cd = cores[core]
        ss, msk, val = cd["slot_send"], cd["mask"], cd["valid"]
        pp = np.zeros((128, 6, NCHE), np.float32)
        rloc = msk.argmax(2)
        rglob = core * NC_AT + (np.arange(NCHE)[None, :] * CNT + rloc)
        rglob = np.clip(rglob, 0, N_ATOMS - 1)
        pp[:, 0:3, :] = np.where(val[:, None, :],
                                 positions[ss].transpose(0, 2, 1), 0.0)
        pp[:, 3:6, :] = np.where(val[:, None, :],
                                 positions[rglob].transpose(0, 2, 1), 0.0)
        at = np.arange(core * NC_AT, core * NC_AT + NPAD)
        atc = np.clip(at, 0, N_ATOMS - 1)
        apad = (at < N_ATOMS)
        cemb_t = np.where(apad[None, :], cemb[atc].T, 0.0).astype(np.float32)
        base_maps.append(dict(pp=pp, msk=msk.astype(BF16), s2=S2, wrb=WRB,
                              cemb=np.ascontiguousarray(cemb_t)))

    def run(hsrc, w3):
        maps = []
        for core in range(NCORES):
            cd = cores[core]
            hsl = np.where(cd["valid"][:, :, None],
                           hsrc[cd["slot_send"]], 0.0).astype(BF16)
            maps.append(dict(base_maps[core], hs=hsl, w3=w3))
        return bass_utils.run_bass_kernel_spmd(nc, maps,
                                               core_ids=list(range(NCORES)))

    HSC = 1.0 / 16.0
    res1 = run(cemb, w3_pack(W_inv1, np.zeros(256, np.float32), HSC))
    h1 = np.concatenate(
        [res1.results[c]["outh"][:, 0:NC_AT].T for c in range(NCORES)], 0)

    res2 = run(h1, w3_pack(W_inv2, w_out / (HSC * HSC), 1.0))
    e_atom = np.concatenate(
        [res2.results[c]["oute"][0, 0:NC_AT] for c in range(NCORES)], 0)
    e_atom = e_atom + comp_weights[species]

    out = np.zeros(N_STRUCT, np.float32)
    np.add.at(out, structure_ids_np, e_atom)
    return out


# revision 14
# speedup vs baseline: 1.1614x; 1.0236x over previous
import sys
sys.path.insert(0, "/opt/trn_rl_repo")
import numpy as np
import ml_dtypes

BF16 = np.float16

N_ATOMS = 10000
N_SPECIES = 8
N_STRUCT = 8
C = 16
N_BASIS = 8
L_MAX = 3
CUTOFF = 5.0
NCORES = 8
NC_AT = N_ATOMS // NCORES

_prog_cache = {}


def _pack(senders, receivers):
    send = np.asarray(senders).astype(np.int64)
    recv = np.asarray(receivers).astype(np.int64)
    order = np.argsort(recv, kind="stable")
    ss = send[order]
    deg = np.bincount(recv, minlength=N_ATOMS)
    CNT = None
    for c in (8, 7, 6, 5, 4, 3, 2):
        ok = True
        npd = ((NC_AT + c - 1) // c) * c
        for core in range(NCORES):
            d = np.zeros(npd, np.int64)
            d[:NC_AT] = deg[core * NC_AT:(core + 1) * NC_AT]
            if d.reshape(-1, c).sum(1).max() > 128:
                ok = False
                break
        if ok:
            CNT = c
            break
    assert CNT is not None
    NCH = (NC_AT + CNT - 1) // CNT
    NCHE = NCH + (NCH & 1)
    NPAD = CNT * NCH
    starts = np.zeros(N_ATOMS + 1, np.int64)
    starts[1:] = np.cumsum(deg)
    cores = []
    for core in range(NCORES):
        slot_send = np.zeros((128, NCHE), np.int64)
        mask = np.zeros((128, NCHE, CNT), np.float32)
        valid = np.zeros((128, NCHE), bool)
        for k in range(NCH):
            row = 0
            for j in range(CNT):
                r = core * NC_AT + k * CNT + j
                if r >= (core + 1) * NC_AT:
                    continue
                a, b = starts[r], starts[r + 1]
                n = b - a
                slot_send[row:row + n, k] = ss[a:b]
                mask[row:row + n, k, j] = 1.0
                valid[row:row + n, k] = True
                row += n
            assert row <= 128
        cores.append(dict(slot_send=slot_send, mask=mask, valid=valid))
    return CNT, NCH, NCHE, NPAD, cores


def _build(CNT, NCH, NCHE, NPAD):
    import concourse.bass as bass
    import concourse.bacc as bacc
    import concourse.tile as tile
    from concourse import mybir

    f32 = mybir.dt.float32
    bf16 = mybir.dt.float16
    ALU = mybir.AluOpType
    AF = mybir.ActivationFunctionType

    NPADE = CNT * NCHE + CNT    # atom slots incl. pad chunk + view slack
    F = 16 * CNT                # (c, j) cols per chunk / pair
    PAIRS = NCHE // 2
    BP = max(1, 512 // F)       # pairs per block (PSUM bank = 512 f32)

    nc = bacc.Bacc("TRN2", target_bir_lowering=False, debug=False,
                   num_devices=NCORES)
    PP_d = nc.dram_tensor("pp", [128, 6, NCHE], f32, kind="ExternalInput").ap()
    MS_d = nc.dram_tensor("msk", [128, NCHE, CNT], bf16,
                          kind="ExternalInput").ap()
    HS_d = nc.dram_tensor("hs", [128, NCHE, 16], bf16,
                          kind="ExternalInput").ap()
    S2_d = nc.dram_tensor("s2", [128, 32], bf16, kind="ExternalInput").ap()
    W3_d = nc.dram_tensor("w3", [16, 16 * 17], bf16, kind="ExternalInput").ap()
    WR_d = nc.dram_tensor("wrb", [128, 8, 16], bf16, kind="ExternalInput").ap()
    CE_d = nc.dram_tensor("cemb", [16, NPAD], f32, kind="ExternalInput").ap()
    OUTH_d = nc.dram_tensor("outh", [16, NPAD], f32, kind="ExternalOutput").ap()
    OUTE_d = nc.dram_tensor("oute", [1, NPAD], f32, kind="ExternalOutput").ap()

    with tile.TileContext(nc) as tc:
        with tc.tile_pool(name="main", bufs=1) as pool, \
             tc.tile_pool(name="gp", bufs=3) as gpool, \
             tc.tile_pool(name="asp", bufs=3) as apool, \
             tc.tile_pool(name="pa", bufs=2, space="PSUM") as ppa, \
             tc.tile_pool(name="pi", bufs=2, space="PSUM") as ppi, \
             tc.tile_pool(name="ph", bufs=1, space="PSUM") as pph:
            PP = pool.tile([128, 6, NCHE], f32, tag="pp")
            S2 = pool.tile([128, 32], bf16, tag="s2")
            W3 = pool.tile([16, 16 * 17], bf16, tag="w3")
            WR = pool.tile([128, 8, 16], bf16, tag="wr")
            CE = pool.tile([16, NPAD], f32, tag="ce")
            MS = pool.tile([128, NCHE, CNT], bf16, tag="ms")
            HS = pool.tile([128, NCHE, 16], bf16, tag="hs")
            for t, d in ((PP, PP_d), (S2, S2_d), (W3, W3_d), (WR, WR_d),
                         (CE, CE_d), (MS, MS_d), (HS, HS_d)):
                nc.sync.dma_start(t[:], d[:])

            INV = pool.tile([16, 16, NPADE], bf16, tag="inv")
            TT = nc.vector.tensor_tensor
            TS = nc.vector.tensor_scalar

            def emit_geometry(r, c0, nk):
                RV = pool.tile([128, 3, nk], f32, tag=f"rv{r}", name=f"rv{r}")
                U = pool.tile([128, 3, nk], f32, tag=f"u{r}", name=f"u{r}")
                SC = pool.tile([128, 12, nk], f32, tag=f"sc{r}", name=f"sc{r}")
                SHB = pool.tile([128, 16, nk], bf16, tag=f"shb{r}",
                                name=f"shb{r}")
                BS = pool.tile([128, 8, nk], bf16, tag=f"bs{r}", name=f"bs{r}")
                RR = pool.tile([128, nk, 16], bf16, tag=f"rr{r}", name=f"rr{r}")
                TM = pool.tile([128, nk, 16], bf16, tag=f"tm{r}", name=f"tm{r}")
                PT = pool.tile([128, nk, 64], bf16, tag=f"pt{r}", name=f"pt{r}")

                def sc(i):
                    return SC[:, i, :]

                TT(RV[:], PP[:, 3:6, c0:c0 + nk], PP[:, 0:3, c0:c0 + nk],
                   ALU.subtract)
                x, y, z = RV[:, 0, :], RV[:, 1, :], RV[:, 2, :]
                nc.vector.tensor_mul(sc(0), x, x)
                nc.vector.tensor_mul(sc(1), y, y)
                TT(sc(0), sc(0), sc(1), ALU.add)
                nc.vector.tensor_mul(sc(1), z, z)
                TT(sc(0), sc(0), sc(1), ALU.add)                 # r2
                nc.scalar.activation(sc(1), sc(0), AF.Sqrt)      # r
                nc.vector.tensor_scalar_max(sc(2), sc(1), 1e-6)  # rc
                nc.vector.reciprocal(sc(3), sc(2))               # rinv
                TT(U[:], RV[:],
                   SC[:, 3:4, :].to_broadcast([128, 3, nk]), ALU.mult)
                nc.vector.tensor_scalar_min(sc(6), sc(1), CUTOFF)
                TS(sc(6), sc(6), float(np.pi / CUTOFF), float(-np.pi / 2),
                   ALU.mult, ALU.add)
                nc.scalar.activation(sc(7), sc(6), AF.Sin)
                TS(sc(4), sc(7), -0.5, 0.5, ALU.mult, ALU.add)   # fc
                TT(sc(5), sc(4), sc(3), ALU.mult)
                nc.vector.tensor_scalar_mul(sc(5), sc(5),
                                            float(np.sqrt(2.0 / CUTOFF)))
                x, y, z = U[:, 0, :], U[:, 1, :], U[:, 2, :]
                x2, y2, z2, xy, yz, xz = (sc(i) for i in (6, 7, 8, 9, 10, 11))
                nc.vector.tensor_mul(x2, x, x)
                nc.vector.tensor_mul(y2, y, y)
                nc.vector.tensor_mul(z2, z, z)
                nc.vector.tensor_mul(xy, x, y)
                nc.vector.tensor_mul(yz, y, z)
                nc.vector.tensor_mul(xz, x, z)
                d_, t_ = sc(0), sc(1)

                def shm(m):
                    return SHB[:, m, :]

                nc.vector.memset(shm(0), 0.28209479)
                nc.vector.tensor_scalar_mul(shm(1), y, 0.48860251)
                nc.vector.tensor_scalar_mul(shm(2), z, 0.48860251)
                nc.vector.tensor_scalar_mul(shm(3), x, 0.48860251)
                nc.vector.tensor_scalar_mul(shm(4), xy, 1.09254843)
                nc.vector.tensor_scalar_mul(shm(5), yz, 1.09254843)
                TS(shm(6), z2, 3.0 * 0.31539157, -0.31539157, ALU.mult,
                   ALU.add)
                nc.vector.tensor_scalar_mul(shm(7), xz, 1.09254843)
                TT(d_, x2, y2, ALU.subtract)
                nc.vector.tensor_scalar_mul(shm(8), d_, 0.54627422)
                nc.vector.scalar_tensor_tensor(t_, x2, 3.0, y2, ALU.mult,
                                               ALU.subtract)
                TT(t_, t_, y, ALU.mult)
                nc.vector.tensor_scalar_mul(shm(9), t_, 0.59004359)
                TT(t_, xy, z, ALU.mult)
                nc.vector.tensor_scalar_mul(shm(10), t_, 2.89061144)
                TS(t_, z2, 5.0 * 0.45704579, -0.45704579, ALU.mult, ALU.add)
                TT(shm(11), t_, y, ALU.mult)
                TT(shm(13), t_, x, ALU.mult)
                TS(t_, z2, 5.0 * 0.37317633, -3.0 * 0.37317633, ALU.mult,
                   ALU.add)
                TT(shm(12), t_, z, ALU.mult)
                TT(t_, d_, z, ALU.mult)
                nc.vector.tensor_scalar_mul(shm(14), t_, 1.44530572)
                nc.vector.scalar_tensor_tensor(t_, y2, -3.0, x2, ALU.mult,
                                               ALU.add)
                TT(t_, t_, x, ALU.mult)
                nc.vector.tensor_scalar_mul(shm(15), t_, 0.59004359)

                C2, SA, SB, TP = sc(0), sc(1), sc(3), sc(4)
                TS(sc(6), sc(2), float(np.pi / CUTOFF), float(-np.pi),
                   ALU.mult, ALU.add)
                nc.scalar.activation(sc(7), sc(6), AF.Sin)
                nc.vector.tensor_scalar_mul(SA, sc(7), -1.0)
                TS(sc(6), sc(2), float(np.pi / CUTOFF), float(-np.pi / 2),
                   ALU.mult, ALU.add)
                nc.scalar.activation(sc(7), sc(6), AF.Sin)
                nc.vector.tensor_scalar_mul(C2, sc(7), -2.0)
                for b in range(1, N_BASIS + 1):
                    if b == 1:
                        cur = SA
                    elif b == 2:
                        TT(SB, C2, SA, ALU.mult)
                        cur = SB
                    else:
                        TT(TP, C2, SB if b % 2 else SA, ALU.mult)
                        dst = SA if b % 2 else SB
                        TT(dst, TP, SA if b % 2 else SB, ALU.subtract)
                        cur = dst
                    TT(BS[:, b - 1, :], cur, sc(5), ALU.mult)

                for b in range(N_BASIS):
                    bsb = BS[:, b, :].unsqueeze(2).to_broadcast([128, nk, 16])
                    wrb = WR[:, b, :].unsqueeze(1).to_broadcast([128, nk, 16])
                    if b == 0:
                        TT(RR[:], bsb, wrb, ALU.mult)
                    else:
                        TT(TM[:], bsb, wrb, ALU.mult)
                        TT(RR[:], RR[:], TM[:], ALU.add)

                for l in range(L_MAX + 1):
                    nm = 2 * l + 1
                    sh_v = SHB[:, l * l:l * l + nm, :].rearrange(
                        "p m k -> p k m").unsqueeze(3).to_broadcast(
                        [128, nk, nm, 4])
                    rr_v = RR[:, :, l * 4:l * 4 + 4].unsqueeze(2).to_broadcast(
                        [128, nk, nm, 4])
                    pt_v = PT[:, :, l * l * 4:(l * l + nm) * 4].rearrange(
                        "p k (m n) -> p k m n", n=4)
                    TT(pt_v, sh_v, rr_v, ALU.mult)
                return PT

            def emit_chunks(c0, nk, PT):
                p0 = 0
                npr = nk // 2
                while p0 < npr:
                    nbp = min(BP, npr - p0)
                    nck = 2 * nbp
                    k0 = c0 + 2 * p0
                    G = gpool.tile([128, 2 * BP, 16, CNT], bf16, tag="g",
                                   name="g")
                    nc.gpsimd.tensor_tensor(
                        G[:, 0:nck, :, :],
                        HS[:, k0:k0 + nck, :].unsqueeze(3).to_broadcast(
                            [128, nck, 16, CNT]),
                        MS[:, k0:k0 + nck, :].unsqueeze(2).to_broadcast(
                            [128, nck, 16, CNT]),
                        ALU.mult)
                    pa = ppa.tile([128, 512], f32, tag="pa", name="pa")
                    for p in range(nbp):
                        for h in (0, 1):
                            kk = 2 * p + h
                            nc.tensor.matmul(
                                pa[64 * h:64 * (h + 1), F * p:F * (p + 1)],
                                PT[:, 2 * p0 + kk, :],
                                G[:, kk, :, :],
                                start=True, stop=True)
                    As = apool.tile([128, 512], bf16, tag="as", name="as")
                    nc.scalar.activation(As[:, 0:F * nbp], pa[:, 0:F * nbp],
                                         AF.Square)
                    for h in (0, 1):
                        pi = ppi.tile([16, 512], f32, tag="pi", name="pi")
                        nc.tensor.matmul(pi[:, 0:F * nbp],
                                         S2[:, 16 * h:16 * (h + 1)],
                                         As[:, 0:F * nbp],
                                         start=True, stop=True)
                        src = pi[:, 0:F * nbp].rearrange(
                            "p (q c j) -> p q c j", c=16, j=CNT)
                        a0 = (k0 + h) * CNT
                        dst = INV[:, :, a0:a0 + nbp * 2 * CNT].rearrange(
                            "p c (q r) -> p q c r", r=2 * CNT)[:, :, :, 0:CNT]
                        nc.scalar.copy(dst, src)
                    p0 += nbp

            NR = 2
            RB = ((PAIRS + NR - 1) // NR)
            for r in range(NR):
                pr0 = r * RB
                npr = min(RB, PAIRS - pr0)
                if npr <= 0:
                    continue
                c0 = 2 * pr0
                nk = 2 * npr
                PTr = emit_geometry(r, c0, nk)
                emit_chunks(c0, nk, PTr)

            # W apply: ph[o, a] = sum_c W3[:, (c,:)] @ INV[:, c, a]
            OUTS = pool.tile([16, NPAD], f32, tag="outs")
            GRP = 512
            ngrp = (NPAD + GRP - 1) // GRP
            phs = [pph.tile([17, 512], f32, tag=f"ph{gi}", name=f"ph{gi}")
                   for gi in range(ngrp)]
            for c in range(16):
                for gi in range(ngrp):
                    off = gi * GRP
                    gsz = min(GRP, NPAD - off)
                    nc.tensor.matmul(phs[gi][:, 0:gsz],
                                     W3[:, c * 17:(c + 1) * 17],
                                     INV[:, c, off:off + gsz],
                                     start=(c == 0), stop=(c == 15))
            ES = pool.tile([17, NPAD], f32, tag="es")
            for gi in range(ngrp):
                off = gi * GRP
                gsz = min(GRP, NPAD - off)
                nc.scalar.copy(ES[:, off:off + gsz], phs[gi][:, 0:gsz])
                nc.vector.tensor_mul(OUTS[:, off:off + gsz],
                                     ES[0:16, off:off + gsz],
                                     CE[:, off:off + gsz])
            nc.sync.dma_start(OUTH_d[:], OUTS[:])
            nc.sync.dma_start(OUTE_d[:], ES[16:17, :])
    nc.compile()
    return nc


def kernel(positions, embed, W_rad, W_inv1, W_inv2, w_out, comp_weights,
           senders, receivers, species, structure_ids):
    from concourse import bass_utils

    positions = np.asarray(positions, np.float32)
    embed = np.asarray(embed, np.float32)
    W_rad = np.asarray(W_rad, np.float32)
    W_inv1 = np.asarray(W_inv1, np.float32)
    W_inv2 = np.asarray(W_inv2, np.float32)
    w_out = np.asarray(w_out, np.float32)
    comp_weights = np.asarray(comp_weights, np.float32)
    senders = np.asarray(senders).astype(np.int64)
    receivers = np.asarray(receivers).astype(np.int64)
    species = np.asarray(species).astype(np.int64)
    structure_ids_np = np.asarray(structure_ids).astype(np.int64)

    CNT, NCH, NCHE, NPAD, cores = _pack(senders, receivers)
    key = (CNT, NCH)
    if key not in _prog_cache:
        _prog_cache[key] = _build(CNT, NCH, NCHE, NPAD)
    nc = _prog_cache[key]

    cemb = embed[species]  # [N,16]
    # binary S2 selector (1/sqrt(2l+1) folded into W3)
    S2 = np.zeros((128, 32), np.float32)
    mi = 0
    for l in range(L_MAX + 1):
        for m in range(2 * l + 1):
            for n in range(4):
                for h in (0, 1):
                    S2[h * 64 + mi * 4 + n, h * 16 + l * 4 + n] = 1.0
            mi += 1
    S2 = S2.astype(BF16)
    WRB = np.zeros((8, 16), np.float32)
    for l in range(L_MAX + 1):
        WRB[:, l * 4:(l + 1) * 4] = W_rad[l]
    WRB = np.broadcast_to(WRB[None], (128, 8, 16)).astype(BF16).copy()

    def w3_pack(W, wo, hscale):
        Waug = np.concatenate([W * hscale, wo[:, None]], 1)  # [256,17]
        W3 = np.zeros((16, 16 * 17), np.float32)
        for l in range(L_MAX + 1):
            s = 1.0 / np.sqrt(2.0 * l + 1.0)
            for n in range(4):
                for c in range(16):
                    W3[l * 4 + n, c * 17:(c + 1) * 17] = \
                        Waug[(l * 4 + n) * 16 + c] * s
        return W3.astype(BF16)

    base_maps = []
    for core in range(NCORES):
        cd = cores[core]
        ss, msk, val = cd["slot_send"], cd["mask"], cd["valid"]
        pp = np.zeros((128, 6, NCHE), np.float32)
        rloc = msk.argmax(2)
        rglob = core * NC_AT + (np.arange(NCHE)[None, :] * CNT + rloc)
        rglob = np.clip(rglob, 0, N_ATOMS - 1)
        pp[:, 0:3, :] = np.where(val[:, None, :],
                                 positions[ss].transpose(0, 2, 1), 0.0)
        pp[:, 3:6, :] = np.where(val[:, None, :],
                                 positions[rglob].transpose(0, 2, 1), 0.0)
        at = np.arange(core * NC_AT, core * NC_AT + NPAD)
        atc = np.clip(at, 0, N_ATOMS - 1)
        apad = (at < N_ATOMS)
        cemb_t = np.where(apad[None, :], cemb[atc].T, 0.0).astype(np.float32)
        base_maps.append(dict(pp=pp, msk=msk.astype(BF16), s2=S2, wrb=WRB,
                              cemb=np.ascontiguousarray(cemb_t)))

    def run(hsrc, w3):
        maps = []
        for core in range(NCORES):
            cd = cores[core]
            hsl = np.where(cd["valid"][:, :, None],
                           hsrc[cd["slot_send"]], 0.0).astype(BF16)
            maps.append(dict(base_maps[core], hs=hsl, w3=w3))
        return bass_utils.run_bass_kernel_spmd(nc, maps,
                                               core_ids=list(range(NCORES)))

    HSC = 1.0 / 16.0
    res1 = run(cemb, w3_pack(W_inv1, np.zeros(256, np.float32), HSC))
    h1 = np.concatenate(
        [res1.results[c]["outh"][:, 0:NC_AT].T for c in range(NCORES)], 0)

    res2 = run(h1, w3_pack(W_inv2, w_out / (HSC * HSC), 1.0))
    e_atom = np.concatenate(
        [res2.results[c]["oute"][0, 0:NC_AT] for c in range(NCORES)], 0)
    e_atom = e_atom + comp_weights[species]

    out = np.zeros(N_STRUCT, np.float32)
    np.add.at(out, structure_ids_np, e_atom)
    return out
